# revision 11
# baseline (speedup 1.0000x reference)
"""ALiBi bias application on 8 TRN2 NeuronCores.

out[b,h,i,j] = scores[b,h,i,j] - slope_h * (pos[b,i] - pos[b,j])
             = (scores[b,h,i,j] - slope_h*pos[b,i]) + slope_h*pos[b,j]

Pure streaming elementwise problem (memory-bound). Sharding: flatten
(B,H) -> 32 matrices, core c owns contiguous matrices [4c, 4c+4) —
bias terms are fully local, no collectives. The tiny per-(b,h) bias
vectors (slope*pos) are precomputed on host and laid out to match the
on-device tile mapping; on device each element is touched by exactly
one fused VectorEngine op (scalar_tensor_tensor) between two big DMAs.
"""

import sys

if "/opt/trn_rl_repo" not in sys.path:
    sys.path.insert(0, "/opt/trn_rl_repo")

import numpy as np

import concourse.bacc as bacc
import concourse.bass as bass
import concourse.mybir as mybir
from concourse.bass_utils import run_bass_kernel_spmd
from concourse.tile import TileContext

B, H, S = 2, 16, 2048
NCORES = 8
M_PER_CORE = (B * H) // NCORES  # 4 matrices per core
ROWS_PER_CHUNK = 512  # 4 MiB contiguous DRAM chunk per DMA
K_SUB = ROWS_PER_CHUNK // 128  # rows owned by one partition within a chunk
CHUNKS_PER_MAT = S // ROWS_PER_CHUNK
N_CHUNKS = M_PER_CORE * CHUNKS_PER_MAT
FREE = K_SUB * S  # SBUF free-dim elems per partition per chunk

_F32 = mybir.dt.float32


def _build_graph():
    nc = bacc.Bacc()
    scores_ext = nc.declare_dram_parameter(
        "scores", [M_PER_CORE, S, S], _F32, isOutput=False
    )
    rowv_ext = nc.declare_dram_parameter(
        "rowv", [128, N_CHUNKS * K_SUB], _F32, isOutput=False
    )
    colv_ext = nc.declare_dram_parameter("colv", [M_PER_CORE * S], _F32, isOutput=False)
    out_ext = nc.declare_dram_parameter("out", [M_PER_CORE, S, S], _F32, isOutput=True)

    with TileContext(nc) as tc:
        with (
            tc.tile_pool(name="const", bufs=1) as cpool,
            tc.tile_pool(name="data", bufs=4) as dpool,
        ):
            rowv_sb = cpool.tile([128, N_CHUNKS * K_SUB], _F32)
            colv_sb = cpool.tile([1, M_PER_CORE * S], _F32)
            colb_sb = cpool.tile([128, M_PER_CORE * S], _F32)
            nc.sync.dma_start(out=rowv_sb[:], in_=rowv_ext[:])
            nc.sync.dma_start(out=colv_sb[:], in_=colv_ext[:])
            # Replicate the column-bias row across all 128 partitions on
            # the (otherwise idle) GPSIMD engine instead of shipping a
            # 128x-replicated tensor through the DMA ports.
            for m in range(M_PER_CORE):
                nc.gpsimd.partition_broadcast(
                    colb_sb[:, m * S : (m + 1) * S],
                    colv_sb[0:1, m * S : (m + 1) * S],
                )
            for c in range(N_CHUNKS):
                m = c // CHUNKS_PER_MAT
                r0 = (c % CHUNKS_PER_MAT) * ROWS_PER_CHUNK
                t = dpool.tile([128, FREE], _F32)
                # Contiguous 4 MiB DRAM run -> [128, FREE]: partition p
                # holds rows r0 + K_SUB*p + k (k = 0..K_SUB-1).
                nc.sync.dma_start(
                    out=t[:], in_=scores_ext[m, r0 : r0 + ROWS_PER_CHUNK, :]
                )
                for k in range(K_SUB):
                    col = c * K_SUB + k
                    nc.vector.scalar_tensor_tensor(
                        t[:, k * S : (k + 1) * S],
                        t[:, k * S : (k + 1) * S],
                        rowv_sb[:, col : col + 1],
                        colb_sb[:, m * S : (m + 1) * S],
                        mybir.AluOpType.subtract,
                        mybir.AluOpType.add,
                    )
                nc.scalar.dma_start(
                    out=out_ext[m, r0 : r0 + ROWS_PER_CHUNK, :], in_=t[:]
                )
    nc.compile()
    return nc


def _make_in_maps(scores, positions, token_indices):
    scores = np.ascontiguousarray(np.asarray(scores, dtype=np.float32))
    positions = np.asarray(positions, dtype=np.float32)
    tidx = np.asarray(token_indices).astype(np.int64)

    # slopes: match reference's f32 computation
    slopes = np.exp2((-8.0 * np.arange(1, H + 1) / H).astype(np.float32)).astype(
        np.float64
    )
    pos = positions.astype(np.float64)[tidx]  # [B, S]

    scores_flat = scores.reshape(B * H, S, S)
    p = np.arange(128)

    in_maps = []
    for core in range(NCORES):
        ms = np.arange(core * M_PER_CORE, (core + 1) * M_PER_CORE)
        bs, hs = ms // H, ms % H
        # colv[m_loc*S + f] = slope_m * pos[b_m, f]  (broadcast on device)
        # rowv[p, c*K_SUB + k] = slope_m * pos[b_m, r0 + K_SUB*p + k]
        colv = (slopes[hs][:, None] * pos[bs]).astype(np.float32).reshape(-1)
        rowv = np.empty((128, N_CHUNKS * K_SUB), dtype=np.float32)
        for c in range(N_CHUNKS):
            m_loc = c // CHUNKS_PER_MAT
            r0 = (c % CHUNKS_PER_MAT) * ROWS_PER_CHUNK
            for k in range(K_SUB):
                rows = r0 + K_SUB * p + k
                rowv[:, c * K_SUB + k] = slopes[hs[m_loc]] * pos[bs[m_loc], rows]
        in_maps.append(
            {
                "scores": scores_flat[core * M_PER_CORE : (core + 1) * M_PER_CORE],
                "rowv": rowv,
                "colv": colv,
            }
        )
    return in_maps


def _run(scores, positions, token_indices, trace=False):
    in_maps = _make_in_maps(scores, positions, token_indices)
    nc = _build_graph()
    res = run_bass_kernel_spmd(nc, in_maps, core_ids=list(range(NCORES)), trace=trace)
    outs = [res.results[i]["out"] for i in range(NCORES)]
    full = np.concatenate(outs, axis=0).reshape(B, H, S, S)
    return full, res


def kernel(scores, positions, token_indices):
    full, _ = _run(scores, positions, token_indices, trace=False)
    return full


# revision 16
# speedup vs baseline: 1.0091x; 1.0091x over previous
"""ALiBi bias application on 8 TRN2 NeuronCores.

out[b,h,i,j] = scores[b,h,i,j] - slope_h * (pos[b,i] - pos[b,j])
             = (scores[b,h,i,j] - slope_h*pos[b,i]) + slope_h*pos[b,j]

Pure streaming elementwise problem (memory-bound). Sharding: flatten
(B,H) -> 32 matrices, core c owns contiguous matrices [4c, 4c+4) —
bias terms are fully local, no collectives. The tiny per-(b,h) bias
vectors (slope*pos) are precomputed on host and laid out to match the
on-device tile mapping; on device each element is touched by exactly
one fused VectorEngine op (scalar_tensor_tensor) between two big DMAs.
"""

import sys

if "/opt/trn_rl_repo" not in sys.path:
    sys.path.insert(0, "/opt/trn_rl_repo")

import numpy as np

import concourse.bacc as bacc
import concourse.bass as bass
import concourse.mybir as mybir
from concourse.bass_utils import run_bass_kernel_spmd
from concourse.tile import TileContext

B, H, S = 2, 16, 2048
NCORES = 8
M_PER_CORE = (B * H) // NCORES  # 4 matrices per core
ROWS_PER_CHUNK = 512  # 4 MiB contiguous DRAM chunk per DMA
K_SUB = ROWS_PER_CHUNK // 128  # rows owned by one partition within a chunk
CHUNKS_PER_MAT = S // ROWS_PER_CHUNK
N_CHUNKS = M_PER_CORE * CHUNKS_PER_MAT
FREE = K_SUB * S  # SBUF free-dim elems per partition per chunk

_F32 = mybir.dt.float32


def _build_graph():
    nc = bacc.Bacc()
    scores_ext = nc.declare_dram_parameter(
        "scores", [M_PER_CORE, S, S], _F32, isOutput=False
    )
    # rowv packed with the per-matrix slope columns: [128, N_CHUNKS*K_SUB + M]
    rowv_ext = nc.declare_dram_parameter(
        "rowv", [128, N_CHUNKS * K_SUB + M_PER_CORE], _F32, isOutput=False
    )
    # this core's batch pos, replicated across partitions: [128, S] (1 MiB)
    # (every matrix of one core shares a batch: 4 consecutive (b,h) pairs
    # never straddle a batch boundary since H % M_PER_CORE == 0)
    posb_ext = nc.declare_dram_parameter("posb", [128, S], _F32, isOutput=False)
    out_ext = nc.declare_dram_parameter("out", [M_PER_CORE, S, S], _F32, isOutput=True)

    with TileContext(nc) as tc:
        with (
            tc.tile_pool(name="const", bufs=1) as cpool,
            tc.tile_pool(name="data", bufs=4) as dpool,
        ):
            rowv_sb = cpool.tile([128, N_CHUNKS * K_SUB + M_PER_CORE], _F32)
            posb_sb = cpool.tile([128, S], _F32)
            colb_sb = cpool.tile([128, M_PER_CORE * S], _F32)
            nc.sync.dma_start(out=rowv_sb[:], in_=rowv_ext[:])
            nc.sync.dma_start(out=posb_sb[:], in_=posb_ext[:])
            # colb_m = slope_m * pos[b] on the (otherwise idle)
            # ScalarEngine — ships pos once instead of a slope-scaled
            # copy per matrix. slope_m comes from data (the packed rowv
            # columns) since it differs per core under SPMD.
            for m in range(M_PER_CORE):
                scol = N_CHUNKS * K_SUB + m
                nc.scalar.activation(
                    colb_sb[:, m * S : (m + 1) * S],
                    posb_sb[:, :],
                    mybir.ActivationFunctionType.Copy,
                    scale=rowv_sb[:, scol : scol + 1],
                )
            for c in range(N_CHUNKS):
                m = c // CHUNKS_PER_MAT
                r0 = (c % CHUNKS_PER_MAT) * ROWS_PER_CHUNK
                t = dpool.tile([128, FREE], _F32)
                # Contiguous 4 MiB DRAM run -> [128, FREE]: partition p
                # holds rows r0 + K_SUB*p + k (k = 0..K_SUB-1).
                nc.sync.dma_start(
                    out=t[:], in_=scores_ext[m, r0 : r0 + ROWS_PER_CHUNK, :]
                )
                for k in range(K_SUB):
                    col = c * K_SUB + k
                    nc.vector.scalar_tensor_tensor(
                        t[:, k * S : (k + 1) * S],
                        t[:, k * S : (k + 1) * S],
                        rowv_sb[:, col : col + 1],
                        colb_sb[:, m * S : (m + 1) * S],
                        mybir.AluOpType.subtract,
                        mybir.AluOpType.add,
                    )
                nc.scalar.dma_start(
                    out=out_ext[m, r0 : r0 + ROWS_PER_CHUNK, :], in_=t[:]
                )
    nc.compile()
    return nc


def _make_in_maps(scores, positions, token_indices):
    scores = np.ascontiguousarray(np.asarray(scores, dtype=np.float32))
    positions = np.asarray(positions, dtype=np.float32)
    tidx = np.asarray(token_indices).astype(np.int64)

    # slopes: match reference's f32 computation
    slopes = np.exp2((-8.0 * np.arange(1, H + 1) / H).astype(np.float32)).astype(
        np.float64
    )
    pos = positions.astype(np.float64)[tidx]  # [B, S]

    scores_flat = scores.reshape(B * H, S, S)
    p = np.arange(128)

    in_maps = []
    for core in range(NCORES):
        ms = np.arange(core * M_PER_CORE, (core + 1) * M_PER_CORE)
        bs, hs = ms // H, ms % H
        # rowv[p, c*K_SUB + k] = slope_m * pos[b_m, r0 + K_SUB*p + k],
        # followed by M_PER_CORE slope columns (device scales pos by these)
        rowv = np.empty((128, N_CHUNKS * K_SUB + M_PER_CORE), dtype=np.float32)
        for c in range(N_CHUNKS):
            m_loc = c // CHUNKS_PER_MAT
            r0 = (c % CHUNKS_PER_MAT) * ROWS_PER_CHUNK
            for k in range(K_SUB):
                rows = r0 + K_SUB * p + k
                rowv[:, c * K_SUB + k] = slopes[hs[m_loc]] * pos[bs[m_loc], rows]
        rowv[:, N_CHUNKS * K_SUB :] = slopes[hs].astype(np.float32)[None, :]
        # all matrices of this core share one batch
        posb = np.ascontiguousarray(
            np.broadcast_to(pos[bs[0]].astype(np.float32)[None, :], (128, S))
        )
        in_maps.append(
            {
                "scores": scores_flat[core * M_PER_CORE : (core + 1) * M_PER_CORE],
                "rowv": rowv,
                "posb": posb,
            }
        )
    return in_maps


def _run(scores, positions, token_indices, trace=False):
    in_maps = _make_in_maps(scores, positions, token_indices)
    nc = _build_graph()
    res = run_bass_kernel_spmd(nc, in_maps, core_ids=list(range(NCORES)), trace=trace)
    outs = [res.results[i]["out"] for i in range(NCORES)]
    full = np.concatenate(outs, axis=0).reshape(B, H, S, S)
    return full, res


def kernel(scores, positions, token_indices):
    full, _ = _run(scores, positions, token_indices, trace=False)
    return full


# revision 19
# speedup vs baseline: 1.1427x; 1.1324x over previous
"""ALiBi bias application on 8 TRN2 NeuronCores.

out[b,h,i,j] = scores[b,h,i,j] - slope_h * (pos[b,i] - pos[b,j])
             = (scores[b,h,i,j] - slope_h*pos[b,i]) + slope_h*pos[b,j]

Pure streaming elementwise problem (memory-bound). Sharding: flatten
(B,H) -> 32 matrices, core c owns contiguous matrices [4c, 4c+4) —
bias terms are fully local, no collectives. The tiny per-(b,h) bias
vectors (slope*pos) are precomputed on host and laid out to match the
on-device tile mapping; on device each element is touched by exactly
one fused VectorEngine op (scalar_tensor_tensor) between two big DMAs.
"""

import sys

if "/opt/trn_rl_repo" not in sys.path:
    sys.path.insert(0, "/opt/trn_rl_repo")

import numpy as np

import concourse.bacc as bacc
import concourse.bass as bass
import concourse.mybir as mybir
from concourse.bass_utils import run_bass_kernel_spmd
from concourse.tile import TileContext

B, H, S = 2, 16, 2048
NCORES = 8
M_PER_CORE = (B * H) // NCORES  # 4 matrices per core
ROWS_PER_CHUNK = 512  # 4 MiB contiguous DRAM chunk per DMA
K_SUB = ROWS_PER_CHUNK // 128  # rows owned by one partition within a chunk
CHUNKS_PER_MAT = S // ROWS_PER_CHUNK
N_CHUNKS = M_PER_CORE * CHUNKS_PER_MAT
FREE = K_SUB * S  # SBUF free-dim elems per partition per chunk

_F32 = mybir.dt.float32


def _build_graph():
    nc = bacc.Bacc()
    scores_ext = nc.declare_dram_parameter(
        "scores", [M_PER_CORE, S, S], _F32, isOutput=False
    )
    # colb ([128, M_PER_CORE*S]) and rowv ([128, N_CHUNKS*K_SUB]) packed
    # side by side — a single DMA/semaphore keeps downstream compute ops
    # within the per-instruction sync-wait limit (1 wait per instruction).
    bias_ext = nc.declare_dram_parameter(
        "bias", [128, M_PER_CORE * S + N_CHUNKS * K_SUB], _F32, isOutput=False
    )
    out_ext = nc.declare_dram_parameter("out", [M_PER_CORE, S, S], _F32, isOutput=True)
    ROW0 = M_PER_CORE * S  # column offset of rowv within bias

    with TileContext(nc) as tc:
        with (
            tc.tile_pool(name="const", bufs=1) as cpool,
            tc.tile_pool(name="data", bufs=4) as dpool,
        ):
            bias_sb = cpool.tile([128, M_PER_CORE * S + N_CHUNKS * K_SUB], _F32)
            nc.sync.dma_start(out=bias_sb[:], in_=bias_ext[:])
            for c in range(N_CHUNKS):
                m = c // CHUNKS_PER_MAT
                r0 = (c % CHUNKS_PER_MAT) * ROWS_PER_CHUNK
                t = dpool.tile([128, FREE], _F32)
                # Contiguous 4 MiB DRAM run -> [128, FREE]: partition p
                # holds rows r0 + K_SUB*p + k (k = 0..K_SUB-1).
                nc.sync.dma_start(
                    out=t[:], in_=scores_ext[m, r0 : r0 + ROWS_PER_CHUNK, :]
                )
                for k in range(K_SUB):
                    col = ROW0 + c * K_SUB + k
                    nc.vector.scalar_tensor_tensor(
                        t[:, k * S : (k + 1) * S],
                        t[:, k * S : (k + 1) * S],
                        bias_sb[:, col : col + 1],
                        bias_sb[:, m * S : (m + 1) * S],
                        mybir.AluOpType.subtract,
                        mybir.AluOpType.add,
                    )
                nc.scalar.dma_start(
                    out=out_ext[m, r0 : r0 + ROWS_PER_CHUNK, :], in_=t[:]
                )
    nc.compile()
    return nc


def _make_in_maps(scores, positions, token_indices):
    scores = np.ascontiguousarray(np.asarray(scores, dtype=np.float32))
    positions = np.asarray(positions, dtype=np.float32)
    tidx = np.asarray(token_indices).astype(np.int64)

    # slopes: match reference's f32 computation
    slopes = np.exp2((-8.0 * np.arange(1, H + 1) / H).astype(np.float32)).astype(
        np.float64
    )
    pos = positions.astype(np.float64)[tidx]  # [B, S]

    scores_flat = scores.reshape(B * H, S, S)
    p = np.arange(128)

    in_maps = []
    for core in range(NCORES):
        ms = np.arange(core * M_PER_CORE, (core + 1) * M_PER_CORE)
        bs, hs = ms // H, ms % H
        # bias[:, :M*S]: colb[p, m_loc*S + f] = slope_m * pos[b_m, f]
        # bias[:, M*S:]: rowv[p, c*K_SUB + k] = slope_m * pos[b_m, r0 + K_SUB*p + k]
        bias = np.empty((128, M_PER_CORE * S + N_CHUNKS * K_SUB), dtype=np.float32)
        colv = (slopes[hs][:, None] * pos[bs]).astype(np.float32)  # [M_PER_CORE, S]
        bias[:, : M_PER_CORE * S] = colv.reshape(1, M_PER_CORE * S)
        for c in range(N_CHUNKS):
            m_loc = c // CHUNKS_PER_MAT
            r0 = (c % CHUNKS_PER_MAT) * ROWS_PER_CHUNK
            for k in range(K_SUB):
                rows = r0 + K_SUB * p + k
                bias[:, M_PER_CORE * S + c * K_SUB + k] = (
                    slopes[hs[m_loc]] * pos[bs[m_loc], rows]
                )
        in_maps.append(
            {
                "scores": scores_flat[core * M_PER_CORE : (core + 1) * M_PER_CORE],
                "bias": bias,
            }
        )
    return in_maps


def _run(scores, positions, token_indices, trace=False):
    in_maps = _make_in_maps(scores, positions, token_indices)
    nc = _build_graph()
    res = run_bass_kernel_spmd(nc, in_maps, core_ids=list(range(NCORES)), trace=trace)
    outs = [res.results[i]["out"] for i in range(NCORES)]
    full = np.concatenate(outs, axis=0).reshape(B, H, S, S)
    return full, res


def kernel(scores, positions, token_indices):
    full, _ = _run(scores, positions, token_indices, trace=False)
    return full


# revision 21
# speedup vs baseline: 1.1464x; 1.0033x over previous
"""ALiBi bias application on 8 TRN2 NeuronCores.

out[b,h,i,j] = scores[b,h,i,j] - slope_h * (pos[b,i] - pos[b,j])
             = (scores[b,h,i,j] - slope_h*pos[b,i]) + slope_h*pos[b,j]

Pure streaming elementwise problem (memory-bound). Sharding: flatten
(B,H) -> 32 matrices, core c owns contiguous matrices [4c, 4c+4) —
bias terms are fully local, no collectives. The tiny per-(b,h) bias
vectors (slope*pos) are precomputed on host and laid out to match the
on-device tile mapping; on device each element is touched by exactly
one fused VectorEngine op (scalar_tensor_tensor) between two big DMAs.
"""

import sys

if "/opt/trn_rl_repo" not in sys.path:
    sys.path.insert(0, "/opt/trn_rl_repo")

import numpy as np

import concourse.bacc as bacc
import concourse.bass as bass
import concourse.mybir as mybir
from concourse.bass_utils import run_bass_kernel_spmd
from concourse.tile import TileContext

B, H, S = 2, 16, 2048
NCORES = 8
M_PER_CORE = (B * H) // NCORES  # 4 matrices per core
ROWS_PER_CHUNK = 512  # 4 MiB contiguous DRAM chunk per DMA
K_SUB = ROWS_PER_CHUNK // 128  # rows owned by one partition within a chunk
CHUNKS_PER_MAT = S // ROWS_PER_CHUNK
N_CHUNKS = M_PER_CORE * CHUNKS_PER_MAT
FREE = K_SUB * S  # SBUF free-dim elems per partition per chunk

_F32 = mybir.dt.float32


def _build_graph():
    nc = bacc.Bacc()
    scores_ext = nc.declare_dram_parameter(
        "scores", [M_PER_CORE, S, S], _F32, isOutput=False
    )
    # colb ([128, M_PER_CORE*S]) and rowv ([128, N_CHUNKS*K_SUB]) packed
    # side by side — a single DMA/semaphore keeps downstream compute ops
    # within the per-instruction sync-wait limit (1 wait per instruction).
    bias_ext = nc.declare_dram_parameter(
        "bias", [128, M_PER_CORE * S + N_CHUNKS * K_SUB], _F32, isOutput=False
    )
    out_ext = nc.declare_dram_parameter("out", [M_PER_CORE, S, S], _F32, isOutput=True)
    ROW0 = M_PER_CORE * S  # column offset of rowv within bias

    with TileContext(nc) as tc:
        with (
            tc.tile_pool(name="const", bufs=1) as cpool,
            tc.tile_pool(name="data", bufs=4) as dpool,
        ):
            bias_sb = cpool.tile([128, M_PER_CORE * S + N_CHUNKS * K_SUB], _F32)
            nc.sync.dma_start(out=bias_sb[:], in_=bias_ext[:])
            for c in range(N_CHUNKS):
                m = c // CHUNKS_PER_MAT
                r0 = (c % CHUNKS_PER_MAT) * ROWS_PER_CHUNK
                t = dpool.tile([128, FREE], _F32)
                # Contiguous 4 MiB DRAM run -> [128, FREE]: partition p
                # holds rows r0 + K_SUB*p + k (k = 0..K_SUB-1).
                nc.sync.dma_start(
                    out=t[:], in_=scores_ext[m, r0 : r0 + ROWS_PER_CHUNK, :]
                )
                for k in range(K_SUB):
                    col = ROW0 + c * K_SUB + k
                    nc.vector.scalar_tensor_tensor(
                        t[:, k * S : (k + 1) * S],
                        t[:, k * S : (k + 1) * S],
                        bias_sb[:, col : col + 1],
                        bias_sb[:, m * S : (m + 1) * S],
                        mybir.AluOpType.subtract,
                        mybir.AluOpType.add,
                    )
                nc.scalar.dma_start(
                    out=out_ext[m, r0 : r0 + ROWS_PER_CHUNK, :], in_=t[:]
                )
    nc.compile()
    return nc


def _make_in_maps(scores, positions, token_indices):
    scores = np.ascontiguousarray(np.asarray(scores, dtype=np.float32))
    positions = np.asarray(positions, dtype=np.float32)
    tidx = np.asarray(token_indices).astype(np.int64)

    # slopes: match reference's f32 computation
    slopes = np.exp2((-8.0 * np.arange(1, H + 1) / H).astype(np.float32)).astype(
        np.float64
    )
    pos = positions.astype(np.float64)[tidx]  # [B, S]

    scores_flat = scores.reshape(B * H, S, S)
    p = np.arange(128)

    in_maps = []
    for core in range(NCORES):
        ms = np.arange(core * M_PER_CORE, (core + 1) * M_PER_CORE)
        bs, hs = ms // H, ms % H
        # bias[:, :M*S]: colb[p, m_loc*S + f] = slope_m * pos[b_m, f]
        # bias[:, M*S:]: rowv[p, c*K_SUB + k] = slope_m * pos[b_m, r0 + K_SUB*p + k]
        bias = np.empty((128, M_PER_CORE * S + N_CHUNKS * K_SUB), dtype=np.float32)
        colv = (slopes[hs][:, None] * pos[bs]).astype(np.float32)  # [M_PER_CORE, S]
        bias[:, : M_PER_CORE * S] = colv.reshape(1, M_PER_CORE * S)
        for c in range(N_CHUNKS):
            m_loc = c // CHUNKS_PER_MAT
            r0 = (c % CHUNKS_PER_MAT) * ROWS_PER_CHUNK
            for k in range(K_SUB):
                rows = r0 + K_SUB * p + k
                bias[:, M_PER_CORE * S + c * K_SUB + k] = (
                    slopes[hs[m_loc]] * pos[bs[m_loc], rows]
                )
        in_maps.append(
            {
                "scores": scores_flat[core * M_PER_CORE : (core + 1) * M_PER_CORE],
                "bias": bias,
            }
        )
    return in_maps


def _run(scores, positions, token_indices, trace=False, reps=1):
    in_maps = _make_in_maps(scores, positions, token_indices)
    nc = _build_graph()
    res = run_bass_kernel_spmd(nc, in_maps, core_ids=list(range(NCORES)), trace=trace)
    times = [res.exec_time_ns]
    for _ in range(reps - 1):
        r2 = run_bass_kernel_spmd(
            nc, in_maps, core_ids=list(range(NCORES)), trace=trace
        )
        times.append(r2.exec_time_ns)
    outs = [res.results[i]["out"] for i in range(NCORES)]
    full = np.concatenate(outs, axis=0).reshape(B, H, S, S)
    return full, res, times


def kernel(scores, positions, token_indices):
    full, _, _ = _run(scores, positions, token_indices, trace=False)
    return full
